# revision 13
# baseline (speedup 1.0000x reference)
"""Trainium2 Bass kernel for nn_Attention_11046655885816.

Full inputs in, full output out. The wall-clock of run_bass_kernel_spmd
is dominated by host<->device transfer over the axon tunnel (~65 MB/s
up, ~38 MB/s down) plus per-call jit lowering, so the kernel is built
to minimize moved bytes:

  * Every input byte is uploaded exactly ONCE: the true-length columns
    of Q/K/V (per batch, padded to 128) and the full weights are packed
    into one fp16 stream, column-sliced into 8 equal chunks (one per
    core, ~4 MB each), and AllGathered device-side over NeuronLink.
  * SPMD cores all run the same program, so per-core data routing uses
    {0,1} selector inputs: each core materializes its (batch,
    head-group) xq/xk/xv/W slices from the gathered stream with
    DMA + multiply-by-selector + accumulate. Wrong-batch pieces are
    multiplied by 0; columns no piece covers stay memset-0.
  * V-masking (zero rows past V_len) moves on-device (per-partition
    scale by the key mask at v-arena assembly), and the softmax divide
    happens on-device too, so the output is fp16 [LQ, 512] per core.
  * jax's persistent compilation cache makes the per-call XLA+NEFF
    compile a disk hit (the fresh jit closure inside run_bass_via_pjrt
    otherwise recompiles every call).

Attention core (per core = one batch, 8 heads) is unchanged from the
working baseline: qT/kT head-major [64*NH, L] fp16 arenas so scores
need no transposes; v_aug carries a kmask column so one AV matmul
accumulation yields numerator and denominator; ScalarE exponentiates
score PSUM quads straight to bf16 T tiles (no max-subtraction needed:
scores are O(+-60) and exp stays in range; masked keys contribute
exactly zero via the zeroed v rows + mask column).
"""

import os
import numpy as np
import ml_dtypes

B, L, D = 4, 2048, 1024
H, DH = 16, 64
NH = 8                      # heads per core (2 head-groups x 4 batches)
EH = NH * DH                # 512
ND = D // 128

_nc_cache = {}
LAST_EXEC_NS = None
LAST_SPMD_WALL_NS = None
LAST_RESULT = None

_JAX_CACHE_DIR = os.path.expanduser("~/.cache/bass_jax_cache")


def _setup_jax_cache():
    import jax

    os.environ.setdefault("JAX_COMPILATION_CACHE_DIR", _JAX_CACHE_DIR)
    for k, v in [
        ("jax_compilation_cache_dir", _JAX_CACHE_DIR),
        ("jax_persistent_cache_min_compile_time_secs", 0.0),
        ("jax_persistent_cache_min_entry_size_bytes", 0),
    ]:
        try:
            jax.config.update(k, v)
        except Exception:
            pass


def _ceil128(n):
    return ((int(n) + 127) // 128) * 128


def _layout(Lqs, Lks):
    """Packed-stream column offsets for the given per-batch lengths."""
    off = {}
    g = 0
    off["q"] = []
    for w in Lqs:
        off["q"].append(g)
        g += w
    off["k"] = []
    for w in Lks:
        off["k"].append(g)
        g += w
    off["v"] = []
    for w in Lks:
        off["v"].append(g)
        g += w
    off["wq"], off["wk"], off["wv"] = g, g + D, g + 2 * D
    g += 3 * D
    ch = ((g + 8 * 128 - 1) // (8 * 128)) * 128
    return off, g, ch


def _build(cfg):
    import concourse.bass as bass
    import concourse.mybir as mybir
    import concourse.tile as tile
    from concourse import bacc

    LQ, LK = cfg["LQ"], cfg["LK"]
    Lqs, Lks = cfg["Lqs"], cfg["Lks"]
    off, TOT, CH = _layout(Lqs, Lks)
    assert LQ % 128 == 0 and LK % 128 == 0
    NEB = EH // 128              # head pairs
    NTK = LK // 128
    VW = DH + 1

    quads = []
    t = 0
    while t < NTK:
        n = min(4, NTK - t)
        quads.append((t, n))
        t += n

    fp16 = mybir.dt.float16
    bf16 = mybir.dt.bfloat16
    f32 = mybir.dt.float32

    # per-head-pair arena strides padded to 8 KiB (odd-4KiB-offset matmul
    # operands returned corrupted scores on HW; see baseline)
    LKS = ((LK * 2 + 8191) // 8192) * 4096
    LQS = ((LQ * 2 + 8191) // 8192) * 4096

    nc = bacc.Bacc(
        "TRN2", target_bir_lowering=False, debug=False, num_devices=8
    )

    # output stream packing: core c=2b+hg owns stream rows
    # [S[c], S[c]+Lqs[b]); ReduceScatter hands core c slot rows
    # [c*SR, (c+1)*SR) so only ~sum(Lqs)*2 rows ever cross the tunnel.
    core_lq = [Lqs[c // 2] for c in range(8)]
    S = [0]
    for c in range(8):
        S.append(S[-1] + core_lq[c])
    STREAM = S[8]
    SR = ((STREAM + 8 * 128 - 1) // (8 * 128)) * 128

    xin = nc.dram_tensor("xin", [D, CH], fp16, kind="ExternalInput").ap()
    sel = nc.dram_tensor("sel", [128, 24], f32, kind="ExternalInput").ap()
    km = nc.dram_tensor("km", [128, NTK * NH], bf16, kind="ExternalInput").ap()
    outp = nc.dram_tensor("outp", [SR, EH], fp16, kind="ExternalOutput").ap()

    with tile.TileContext(nc, trace_sim=False) as tc:
        with (
            tc.tile_pool(name="dram", bufs=1, space="DRAM") as dram,
            tc.tile_pool(name="xc", bufs=3) as xc_pool,
            tc.tile_pool(name="win", bufs=1) as win_pool,
            tc.tile_pool(name="proj", bufs=1) as proj_pool,
            tc.tile_pool(name="mat", bufs=4) as mat_pool,
            tc.tile_pool(name="acc", bufs=3) as acc_pool,
            tc.tile_pool(name="tsb", bufs=6) as t_pool,
            tc.tile_pool(name="osb", bufs=8) as o_pool,
            tc.tile_pool(name="rsb", bufs=8) as r_pool,
            tc.tile_pool(name="ps", bufs=2, space="PSUM") as pp_pool,
            tc.tile_pool(name="pav", bufs=2, space="PSUM") as pav_pool,
            tc.tile_pool(name="pj", bufs=2, space="PSUM") as pj_pool,
        ):
            # ---- gather every core's packed chunk ----
            bx = dram.tile([D, CH], fp16, tag="bx")
            gx = dram.tile([8 * D, CH], fp16, tag="gx")
            nc.gpsimd.dma_start(bx[:], xin)
            nc.gpsimd.collective_compute(
                "AllGather",
                mybir.AluOpType.bypass,
                replica_groups=[list(range(8))],
                ins=[bx[:].opt()],
                outs=[gx[:].opt()],
            )

            xqb = dram.tile([D, LQ], fp16, tag="xqb")
            xkb = dram.tile([D, LK], fp16, tag="xkb")
            xvb = dram.tile([D, LK], fp16, tag="xvb")
            xo = dram.tile([LQ, EH], fp16, tag="xo")
            contrib = dram.tile([8 * SR, EH], fp16, tag="contrib")
            rsout = dram.tile([SR, EH], fp16, tag="rsout")

            sel_sb = win_pool.tile([128, 24], f32, tag="sel")
            km_sb = win_pool.tile([128, NTK * NH], bf16, tag="kms")
            nc.sync.dma_start(sel_sb[:], sel)
            nc.sync.dma_start(km_sb[:], km)
            # f32 copy of the per-key-tile mask (activation scale must be f32)
            kmf = win_pool.tile([128, NTK], f32, tag="kmf")
            nc.vector.tensor_copy(
                kmf[:],
                km_sb[:].rearrange("p (t h) -> p t h", h=NH)[:, :, 0],
            )

            # ---- persistent SBUF arenas ----
            wq_sb = win_pool.tile([128, ND * EH], fp16, tag="wq")
            wk_sb = win_pool.tile([128, ND * EH], fp16, tag="wk")
            wv_sb = win_pool.tile([128, ND * EH], fp16, tag="wv")
            qt_sb = proj_pool.tile([128, NEB * LQS], fp16, tag="qt")
            kt_sb = proj_pool.tile([128, NEB * LKS], fp16, tag="kt")
            v_sb = proj_pool.tile([128, NTK * NH * VW], bf16, tag="v")
            v4 = v_sb[:].rearrange("p (t h c) -> p t h c", t=NTK, h=NH, c=VW)
            nc.sync.dma_start(
                v4[:, :, :, DH],
                km.rearrange("p (t h) -> p t h", h=NH),
            )

            def materialize(write, width, pieces):
                """Select-accumulate packed-stream pieces into a target.

                write(dt, w0, wl, acc): store acc[:, :wl] at rows
                [dt*128,(dt+1)*128), cols [w0, w0+wl) of the target.
                pieces: (global_col_offset, piece_width, sel_col) — every
                piece targets cols [0, piece_width); the selector picks
                which piece survives on this core.
                """
                for dt in range(ND):
                    for w0 in range(0, width, 512):
                        wl = min(512, width - w0)
                        acc = acc_pool.tile([128, 512], fp16, tag="acc")
                        nc.vector.memset(acc[:, :wl], 0.0)
                        for (goff, pw, sc) in pieces:
                            cw = min(pw - w0, wl)
                            if cw <= 0:
                                continue
                            s = 0
                            while s < cw:
                                g = goff + w0 + s
                                j, lc = divmod(g, CH)
                                sl = min(cw - s, CH - lc)
                                tmp = mat_pool.tile([128, 512], fp16, tag="t")
                                nc.sync.dma_start(
                                    tmp[:, :sl],
                                    gx[j * D + dt * 128 : j * D + (dt + 1) * 128,
                                       lc : lc + sl],
                                )
                                tm = mat_pool.tile([128, 512], fp16, tag="t2")
                                nc.vector.tensor_scalar_mul(
                                    tm[:, :sl], tmp[:, :sl], sel_sb[:, sc : sc + 1]
                                )
                                nc.vector.tensor_tensor(
                                    acc[:, s : s + sl], acc[:, s : s + sl],
                                    tm[:, :sl], mybir.AluOpType.add,
                                )
                                s += sl
                        write(dt, w0, wl, acc)

            def dram_writer(dst):
                def w(dt, w0, wl, acc):
                    nc.sync.dma_start(
                        dst[dt * 128 : (dt + 1) * 128, w0 : w0 + wl], acc[:, :wl]
                    )
                return w

            def sbuf_writer(dst_arena):
                def w(dt, w0, wl, acc):
                    nc.vector.tensor_copy(
                        dst_arena[:, dt * EH + w0 : dt * EH + w0 + wl], acc[:, :wl]
                    )
                return w

            materialize(
                dram_writer(xqb), LQ,
                [(off["q"][b], Lqs[b], b) for b in range(B)],
            )
            materialize(
                dram_writer(xkb), LK,
                [(off["k"][b], Lks[b], 16 + b) for b in range(B)],
            )
            materialize(
                dram_writer(xvb), LK,
                [(off["v"][b], Lks[b], 16 + b) for b in range(B)],
            )
            for wname, arena in (("wq", wq_sb), ("wk", wk_sb), ("wv", wv_sb)):
                materialize(
                    sbuf_writer(arena), EH,
                    [(off[wname] + hg * EH, EH, 4 + hg) for hg in range(2)],
                )

            def stream_x(src):
                def get(lc, w):
                    xc = xc_pool.tile([128, ND * 512], fp16, tag="xc")
                    for dt in range(ND):
                        nc.sync.dma_start(
                            xc[:, dt * 512 : dt * 512 + w],
                            src[dt * 128 : (dt + 1) * 128, lc : lc + w],
                        )
                    return xc
                return get

            get_xv = stream_x(xvb)
            get_xk = stream_x(xkb)
            get_xq = stream_x(xqb)

            # ---- projections ----
            def proj_v():
                for lc in range(0, LK, 512):
                    w = min(512, LK - lc)
                    xcv = get_xv(lc, w)
                    for t4 in range((w + 127) // 128):
                        t = lc // 128 + t4
                        ps = pj_pool.tile([128, 512], f32, tag="pj")
                        for dt in range(ND):
                            nc.tensor.matmul(
                                ps[:, :EH],
                                lhsT=xcv[:, dt * 512 + t4 * 128 : dt * 512 + (t4 + 1) * 128],
                                rhs=wv_sb[:, dt * EH : (dt + 1) * EH],
                                start=(dt == 0),
                                stop=(dt == ND - 1),
                            )
                        # mask rows past V_len (per-partition key mask) so
                        # masked keys contribute exactly zero to the numerator
                        nc.scalar.mul(
                            v4[:, t, :, 0:DH],
                            ps[:, :EH].rearrange("p (h e) -> p h e", h=NH, e=DH),
                            kmf[:, t : t + 1],
                        )

            def proj_kq(eb):
                for lc in range(0, LK, 512):
                    w = min(512, LK - lc)
                    xck = get_xk(lc, w)
                    ps = pj_pool.tile([128, 512], f32, tag="pj")
                    for dt in range(ND):
                        nc.tensor.matmul(
                            ps[:, :w],
                            lhsT=wk_sb[:, dt * EH + eb * 128 : dt * EH + (eb + 1) * 128],
                            rhs=xck[:, dt * 512 : dt * 512 + w],
                            start=(dt == 0),
                            stop=(dt == ND - 1),
                        )
                    nc.vector.tensor_copy(
                        kt_sb[:, eb * LKS + lc : eb * LKS + lc + w], ps[:, :w]
                    )
                for lc in range(0, LQ, 512):
                    w = min(512, LQ - lc)
                    xcq = get_xq(lc, w)
                    ps = pj_pool.tile([128, 512], f32, tag="pj")
                    for dt in range(ND):
                        nc.tensor.matmul(
                            ps[:, :w],
                            lhsT=wq_sb[:, dt * EH + eb * 128 : dt * EH + (eb + 1) * 128],
                            rhs=xcq[:, dt * 512 : dt * 512 + w],
                            start=(dt == 0),
                            stop=(dt == ND - 1),
                        )
                    nc.vector.tensor_copy(
                        qt_sb[:, eb * LQS + lc : eb * LQS + lc + w], ps[:, :w]
                    )

            # ---- attention; projection of the NEXT head pair interleaved ----
            proj_kq(0)
            proj_v()
            for hp in range(NEB):
                hA, hB = 2 * hp, 2 * hp + 1
                for lqs in range(0, LQ, 256):
                    w = min(256, LQ - lqs)
                    nlqb = w // 128
                    tA = t_pool.tile([128, NTK * 256], bf16, tag="t")
                    tB = t_pool.tile([128, NTK * 256], bf16, tag="t")
                    for (t0, tn) in quads:
                        psA = pp_pool.tile([128, 1024], f32, tag="sq")
                        psB = pp_pool.tile([128, 1024], f32, tag="sq")
                        for j in range(tn):
                            tt = t0 + j
                            nc.tensor.matmul(
                                psA[:, j * w : (j + 1) * w],
                                lhsT=kt_sb[0:64, hp * LKS + tt * 128 : hp * LKS + (tt + 1) * 128],
                                rhs=qt_sb[0:64, hp * LQS + lqs : hp * LQS + lqs + w],
                                start=True,
                                stop=True,
                            )
                            nc.tensor.matmul(
                                psB[:, j * w : (j + 1) * w],
                                lhsT=kt_sb[64:128, hp * LKS + tt * 128 : hp * LKS + (tt + 1) * 128],
                                rhs=qt_sb[64:128, hp * LQS + lqs : hp * LQS + lqs + w],
                                start=True,
                                stop=True,
                            )
                        w_all = tn * w
                        nc.scalar.activation(
                            tA[:, t0 * w : t0 * w + w_all], psA[:, :w_all],
                            mybir.ActivationFunctionType.Exp,
                        )
                        nc.scalar.activation(
                            tB[:, t0 * w : t0 * w + w_all], psB[:, :w_all],
                            mybir.ActivationFunctionType.Exp,
                        )
                    for lb in range(nlqb):
                        pavA = pav_pool.tile([128, VW], f32, tag="av")
                        pavB = pav_pool.tile([128, VW], f32, tag="av")
                        for tt in range(NTK):
                            nc.tensor.matmul(
                                pavA[:, 0:VW],
                                lhsT=tA[:, tt * w + lb * 128 : tt * w + lb * 128 + 128],
                                rhs=v4[:, tt, hA, :],
                                start=(tt == 0),
                                stop=(tt == NTK - 1),
                            )
                            nc.tensor.matmul(
                                pavB[:, 0:VW],
                                lhsT=tB[:, tt * w + lb * 128 : tt * w + lb * 128 + 128],
                                rhs=v4[:, tt, hB, :],
                                start=(tt == 0),
                                stop=(tt == NTK - 1),
                            )
                        rA = r_pool.tile([128, 1], f32, tag="r")
                        rB = r_pool.tile([128, 1], f32, tag="r")
                        nc.vector.reciprocal(rA[:, :], pavA[:, DH : DH + 1])
                        nc.vector.reciprocal(rB[:, :], pavB[:, DH : DH + 1])
                        oA = o_pool.tile([128, DH], fp16, tag="o")
                        oB = o_pool.tile([128, DH], fp16, tag="o")
                        nc.scalar.mul(oA[:, :], pavA[:, 0:DH], rA[:, 0:1])
                        nc.scalar.mul(oB[:, :], pavB[:, 0:DH], rB[:, 0:1])
                        ls = lqs + lb * 128
                        nc.sync.dma_start(
                            xo[ls : ls + 128, hA * DH : (hA + 1) * DH], oA[:, :]
                        )
                        nc.sync.dma_start(
                            xo[ls : ls + 128, hB * DH : (hB + 1) * DH], oB[:, :]
                        )
                if hp + 1 < NEB:
                    proj_kq(hp + 1)

            # ---- pack the output stream ----
            # Each core writes its result into every candidate slot, scaled
            # by the one-hot core indicator (data-routing again: SPMD cores
            # can't address by core id). ReduceScatter(add) then leaves core
            # c exactly slot rows [c*SR, (c+1)*SR).
            for cc in range(8):
                for ls in range(0, min(LQ, core_lq[cc]), 128):
                    ot = mat_pool.tile([128, EH], fp16, tag="ot")
                    nc.sync.dma_start(ot[:], xo[ls : ls + 128, :])
                    om = mat_pool.tile([128, EH], fp16, tag="om")
                    nc.vector.tensor_scalar_mul(
                        om[:], ot[:], sel_sb[:, 8 + cc : 9 + cc]
                    )
                    nc.sync.dma_start(
                        contrib[S[cc] + ls : S[cc] + ls + 128, :], om[:]
                    )
            if STREAM < 8 * SR:
                zt = win_pool.tile([128, EH], fp16, tag="zt")
                nc.vector.memset(zt[:], 0.0)
                for r0 in range(STREAM, 8 * SR, 128):
                    nc.sync.dma_start(contrib[r0 : r0 + 128, :], zt[:])
            nc.gpsimd.collective_compute(
                "ReduceScatter",
                mybir.AluOpType.add,
                replica_groups=[list(range(8))],
                ins=[contrib[:].opt()],
                outs=[rsout[:].opt()],
            )
            nc.gpsimd.dma_start(outp, rsout[:])

    nc.compile()
    return nc


def _get_nc(cfg):
    key = (cfg["LQ"], cfg["LK"], cfg["Lqs"], cfg["Lks"])
    if key not in _nc_cache:
        _nc_cache[key] = _build(cfg)
    return _nc_cache[key]


def kernel(Q_seq, K_seq, V_seq, Q_len, V_len, WQ, WK, WV):
    _setup_jax_cache()
    from concourse.bass_utils import run_bass_kernel_spmd

    Q_seq = np.asarray(Q_seq, np.float32)
    K_seq = np.asarray(K_seq, np.float32)
    V_seq = np.asarray(V_seq, np.float32)
    WQ = np.asarray(WQ, np.float32)
    WK = np.asarray(WK, np.float32)
    WV = np.asarray(WV, np.float32)
    q_len = np.asarray(Q_len).reshape(-1).astype(np.int64)
    v_len = np.asarray(V_len).reshape(-1).astype(np.int64)
    assert len(q_len) == B and Q_seq.shape == (B, L, D)

    # V_len == 0 masks every key, which softmax's shift-invariance turns
    # into "no mask"; Q rows past Q_len are zeroed host-side.
    vl = [int(v) if v > 0 else L for v in v_len]
    Lqs = tuple(_ceil128(min(int(q), L)) for q in q_len)
    Lks = tuple(_ceil128(min(v, L)) for v in vl)
    LQ, LK = max(Lqs), max(Lks)
    out = np.zeros((B, L, H * DH), np.float32)
    if LQ == 0:
        return out
    NTK = LK // 128
    cfg = {"LQ": LQ, "LK": LK, "Lqs": Lqs, "Lks": Lks}
    off, TOT, CH = _layout(Lqs, Lks)
    nc = _get_nc(cfg)

    # ---- pack the upload stream (each byte uploaded exactly once) ----
    f16 = np.float16
    bf16 = ml_dtypes.bfloat16
    X = np.zeros((D, 8 * CH), f16)
    for b in range(B):
        if Lqs[b]:
            X[:, off["q"][b] : off["q"][b] + Lqs[b]] = Q_seq[b, : Lqs[b]].T
        X[:, off["k"][b] : off["k"][b] + Lks[b]] = K_seq[b, : Lks[b]].T
        X[:, off["v"][b] : off["v"][b] + Lks[b]] = V_seq[b, : Lks[b]].T
    X[:, off["wq"] : off["wq"] + D] = WQ
    X[:, off["wk"] : off["wk"] + D] = WK
    X[:, off["wv"] : off["wv"] + D] = WV
    # contiguous per-core chunks so the concatenate inside
    # run_bass_via_pjrt is a plain memcpy, not a strided gather
    Xc = [np.ascontiguousarray(X[:, c * CH : (c + 1) * CH]) for c in range(8)]

    in_maps = []
    core_meta = []
    for b in range(B):
        for hg in range(2):
            c = 2 * b + hg
            s = np.zeros((128, 24), np.float32)
            # reference semantics for V_len==0: scores-1e12 underflows all
            # scores equally in fp32, so softmax is UNIFORM over all keys.
            # Zeroing q reproduces that exactly (exp(0)=1 for every key).
            s[:, b] = 0.0 if int(v_len[b]) == 0 else 1.0
            s[:, 16 + b] = 1.0
            s[:, 4 + hg] = 1.0
            s[:, 8 + c] = 1.0
            kmask = (np.arange(LK) < vl[b]).astype(np.float32)
            kmv = np.repeat(
                kmask.reshape(NTK, 128).T[:, :, None], NH, axis=2
            ).reshape(128, NTK * NH)
            in_maps.append({
                "xin": Xc[c],
                "sel": s,
                "km": kmv.astype(bf16),
            })
            core_meta.append((b, hg))

    import time as _time

    trace = os.environ.get("NN_ATT_TRACE") == "1"
    t_spmd = _time.time()
    try:
        res = run_bass_kernel_spmd(
            nc, in_maps, core_ids=list(range(8)), trace=trace,
            **({"trace_cores": list(range(8))} if trace else {}),
        )
    except Exception:
        if not trace:
            raise
        res = run_bass_kernel_spmd(nc, in_maps, core_ids=list(range(8)))
    global LAST_EXEC_NS, LAST_RESULT, LAST_SPMD_WALL_NS
    LAST_SPMD_WALL_NS = int((_time.time() - t_spmd) * 1e9)
    LAST_RESULT = res
    if res.exec_time_ns:
        LAST_EXEC_NS = int(res.exec_time_ns)

    stream = np.concatenate([res.results[c]["outp"] for c in range(8)], axis=0)
    S = 0
    for c, (b, hg) in enumerate(core_meta):
        blk = Lqs[b]
        nq = min(int(q_len[b]), LQ, L)
        if nq > 0:
            out[b, :nq, hg * EH : (hg + 1) * EH] = stream[S : S + nq].astype(
                np.float32
            )
        S += blk
    return out


# revision 15
# speedup vs baseline: 1.1186x; 1.1186x over previous
"""Trainium2 Bass kernel for nn_Attention_11046655885816.

Full inputs in, full output out. The wall-clock of run_bass_kernel_spmd
is dominated by host<->device transfer over the axon tunnel (~65 MB/s
up, ~38 MB/s down) plus per-call jit lowering, so the kernel is built
to minimize moved bytes:

  * Every input byte is uploaded exactly ONCE: the true-length columns
    of Q/K/V (per batch, padded to 128) and the full weights are packed
    into one fp16 stream, column-sliced into 8 equal chunks (one per
    core, ~4 MB each), and AllGathered device-side over NeuronLink.
  * SPMD cores all run the same program, so per-core data routing uses
    {0,1} selector inputs: each core materializes its (batch,
    head-group) xq/xk/xv/W slices from the gathered stream with
    DMA + multiply-by-selector + accumulate. Wrong-batch pieces are
    multiplied by 0; columns no piece covers stay memset-0.
  * V-masking (zero rows past V_len) moves on-device (per-partition
    scale by the key mask at v-arena assembly), and the softmax divide
    happens on-device too, so the output is fp16 [LQ, 512] per core.
  * jax's persistent compilation cache makes the per-call XLA+NEFF
    compile a disk hit (the fresh jit closure inside run_bass_via_pjrt
    otherwise recompiles every call).

Attention core (per core = one batch, 8 heads) is unchanged from the
working baseline: qT/kT head-major [64*NH, L] fp16 arenas so scores
need no transposes; v_aug carries a kmask column so one AV matmul
accumulation yields numerator and denominator; ScalarE exponentiates
score PSUM quads straight to bf16 T tiles (no max-subtraction needed:
scores are O(+-60) and exp stays in range; masked keys contribute
exactly zero via the zeroed v rows + mask column).
"""

import os
import numpy as np
import ml_dtypes

B, L, D = 4, 2048, 1024
H, DH = 16, 64
NH = 8                      # heads per core (2 head-groups x 4 batches)
EH = NH * DH                # 512
ND = D // 128

_nc_cache = {}
LAST_EXEC_NS = None
LAST_SPMD_WALL_NS = None
LAST_RESULT = None

_JAX_CACHE_DIR = os.path.expanduser("~/.cache/bass_jax_cache")


def _setup_jax_cache():
    import jax

    os.environ.setdefault("JAX_COMPILATION_CACHE_DIR", _JAX_CACHE_DIR)
    for k, v in [
        ("jax_compilation_cache_dir", _JAX_CACHE_DIR),
        ("jax_persistent_cache_min_compile_time_secs", 0.0),
        ("jax_persistent_cache_min_entry_size_bytes", 0),
    ]:
        try:
            jax.config.update(k, v)
        except Exception:
            pass


def _ceil128(n):
    return ((int(n) + 127) // 128) * 128


def _layout(Lqs, Lks):
    """Packed-stream column offsets for the given per-batch lengths."""
    off = {}
    g = 0
    off["q"] = []
    for w in Lqs:
        off["q"].append(g)
        g += w
    off["k"] = []
    for w in Lks:
        off["k"].append(g)
        g += w
    off["v"] = []
    for w in Lks:
        off["v"].append(g)
        g += w
    off["wq"], off["wk"], off["wv"] = g, g + D, g + 2 * D
    g += 3 * D
    ch = ((g + 8 * 128 - 1) // (8 * 128)) * 128
    return off, g, ch


def _build(cfg):
    import concourse.bass as bass
    import concourse.mybir as mybir
    import concourse.tile as tile
    from concourse import bacc

    LQ, LK = cfg["LQ"], cfg["LK"]
    Qe, Ke = cfg["Qe"], cfg["Ke"]
    off, TOT, CH = _layout(Qe, Ke)
    assert LQ % 128 == 0 and LK % 128 == 0
    NEB = EH // 128              # head pairs
    NTK = LK // 128
    VW = DH + 1

    quads = []
    t = 0
    while t < NTK:
        n = min(4, NTK - t)
        quads.append((t, n))
        t += n

    fp16 = mybir.dt.float16
    bf16 = mybir.dt.bfloat16
    f32 = mybir.dt.float32

    # per-head-pair arena strides padded to 8 KiB (odd-4KiB-offset matmul
    # operands returned corrupted scores on HW; see baseline)
    LKS = ((LK * 2 + 8191) // 8192) * 4096
    LQS = ((LQ * 2 + 8191) // 8192) * 4096

    nc = bacc.Bacc(
        "TRN2", target_bir_lowering=False, debug=False, num_devices=8
    )

    # output stream packing: core c=2b+hg owns stream rows
    # [S[c], S[c]+Lqs[b]); ReduceScatter hands core c slot rows
    # [c*SR, (c+1)*SR) so only ~sum(Lqs)*2 rows ever cross the tunnel.
    core_lq = [Qe[c // 2] for c in range(8)]
    S = [0]
    for c in range(8):
        S.append(S[-1] + core_lq[c])
    STREAM = S[8]
    SR = (STREAM + 7) // 8

    xin = nc.dram_tensor("xin", [D, CH], fp16, kind="ExternalInput").ap()
    sel = nc.dram_tensor("sel", [128, 24], f32, kind="ExternalInput").ap()
    km = nc.dram_tensor("km", [128, NTK * NH], bf16, kind="ExternalInput").ap()
    outp = nc.dram_tensor("outp", [SR, EH], fp16, kind="ExternalOutput").ap()

    with tile.TileContext(nc, trace_sim=False) as tc:
        with (
            tc.tile_pool(name="dram", bufs=1, space="DRAM") as dram,
            tc.tile_pool(name="xc", bufs=3) as xc_pool,
            tc.tile_pool(name="win", bufs=1) as win_pool,
            tc.tile_pool(name="proj", bufs=1) as proj_pool,
            tc.tile_pool(name="mat", bufs=4) as mat_pool,
            tc.tile_pool(name="acc", bufs=3) as acc_pool,
            tc.tile_pool(name="tsb", bufs=6) as t_pool,
            tc.tile_pool(name="osb", bufs=8) as o_pool,
            tc.tile_pool(name="rsb", bufs=8) as r_pool,
            tc.tile_pool(name="ps", bufs=2, space="PSUM") as pp_pool,
            tc.tile_pool(name="pav", bufs=2, space="PSUM") as pav_pool,
            tc.tile_pool(name="pj", bufs=2, space="PSUM") as pj_pool,
        ):
            # ---- gather every core's packed chunk ----
            bx = dram.tile([D, CH], fp16, tag="bx")
            gx = dram.tile([8 * D, CH], fp16, tag="gx")
            nc.gpsimd.dma_start(bx[:], xin)
            nc.gpsimd.collective_compute(
                "AllGather",
                mybir.AluOpType.bypass,
                replica_groups=[list(range(8))],
                ins=[bx[:].opt()],
                outs=[gx[:].opt()],
            )

            xqb = dram.tile([D, LQ], fp16, tag="xqb")
            xkb = dram.tile([D, LK], fp16, tag="xkb")
            xvb = dram.tile([D, LK], fp16, tag="xvb")
            xo = dram.tile([LQ, EH], fp16, tag="xo")
            contrib = dram.tile([8 * SR, EH], fp16, tag="contrib")
            rsout = dram.tile([SR, EH], fp16, tag="rsout")

            sel_sb = win_pool.tile([128, 24], f32, tag="sel")
            km_sb = win_pool.tile([128, NTK * NH], bf16, tag="kms")
            nc.sync.dma_start(sel_sb[:], sel)
            nc.sync.dma_start(km_sb[:], km)
            # f32 copy of the per-key-tile mask (activation scale must be f32)
            kmf = win_pool.tile([128, NTK], f32, tag="kmf")
            nc.vector.tensor_copy(
                kmf[:],
                km_sb[:].rearrange("p (t h) -> p t h", h=NH)[:, :, 0],
            )

            # ---- persistent SBUF arenas ----
            wq_sb = win_pool.tile([128, ND * EH], fp16, tag="wq")
            wk_sb = win_pool.tile([128, ND * EH], fp16, tag="wk")
            wv_sb = win_pool.tile([128, ND * EH], fp16, tag="wv")
            qt_sb = proj_pool.tile([128, NEB * LQS], fp16, tag="qt")
            kt_sb = proj_pool.tile([128, NEB * LKS], fp16, tag="kt")
            v_sb = proj_pool.tile([128, NTK * NH * VW], bf16, tag="v")
            v4 = v_sb[:].rearrange("p (t h c) -> p t h c", t=NTK, h=NH, c=VW)
            nc.sync.dma_start(
                v4[:, :, :, DH],
                km.rearrange("p (t h) -> p t h", h=NH),
            )

            def materialize(write, width, pieces):
                """Select-accumulate packed-stream pieces into a target.

                write(dt, w0, wl, acc): store acc[:, :wl] at rows
                [dt*128,(dt+1)*128), cols [w0, w0+wl) of the target.
                pieces: (global_col_offset, piece_width, sel_col) — every
                piece targets cols [0, piece_width); the selector picks
                which piece survives on this core.
                """
                for dt in range(ND):
                    for w0 in range(0, width, 512):
                        wl = min(512, width - w0)
                        acc = acc_pool.tile([128, 512], fp16, tag="acc")
                        nc.vector.memset(acc[:, :wl], 0.0)
                        for (goff, pw, sc) in pieces:
                            cw = min(pw - w0, wl)
                            if cw <= 0:
                                continue
                            s = 0
                            while s < cw:
                                g = goff + w0 + s
                                j, lc = divmod(g, CH)
                                sl = min(cw - s, CH - lc)
                                tmp = mat_pool.tile([128, 512], fp16, tag="t")
                                nc.sync.dma_start(
                                    tmp[:, :sl],
                                    gx[j * D + dt * 128 : j * D + (dt + 1) * 128,
                                       lc : lc + sl],
                                )
                                tm = mat_pool.tile([128, 512], fp16, tag="t2")
                                nc.vector.tensor_scalar_mul(
                                    tm[:, :sl], tmp[:, :sl], sel_sb[:, sc : sc + 1]
                                )
                                nc.vector.tensor_tensor(
                                    acc[:, s : s + sl], acc[:, s : s + sl],
                                    tm[:, :sl], mybir.AluOpType.add,
                                )
                                s += sl
                        write(dt, w0, wl, acc)

            def dram_writer(dst):
                def w(dt, w0, wl, acc):
                    nc.sync.dma_start(
                        dst[dt * 128 : (dt + 1) * 128, w0 : w0 + wl], acc[:, :wl]
                    )
                return w

            def sbuf_writer(dst_arena):
                def w(dt, w0, wl, acc):
                    nc.vector.tensor_copy(
                        dst_arena[:, dt * EH + w0 : dt * EH + w0 + wl], acc[:, :wl]
                    )
                return w

            materialize(
                dram_writer(xqb), LQ,
                [(off["q"][b], Qe[b], b) for b in range(B)],
            )
            materialize(
                dram_writer(xkb), LK,
                [(off["k"][b], Ke[b], 16 + b) for b in range(B)],
            )
            materialize(
                dram_writer(xvb), LK,
                [(off["v"][b], Ke[b], 16 + b) for b in range(B)],
            )
            for wname, arena in (("wq", wq_sb), ("wk", wk_sb), ("wv", wv_sb)):
                materialize(
                    sbuf_writer(arena), EH,
                    [(off[wname] + hg * EH, EH, 4 + hg) for hg in range(2)],
                )

            def stream_x(src):
                def get(lc, w):
                    xc = xc_pool.tile([128, ND * 512], fp16, tag="xc")
                    for dt in range(ND):
                        nc.sync.dma_start(
                            xc[:, dt * 512 : dt * 512 + w],
                            src[dt * 128 : (dt + 1) * 128, lc : lc + w],
                        )
                    return xc
                return get

            get_xv = stream_x(xvb)
            get_xk = stream_x(xkb)
            get_xq = stream_x(xqb)

            # ---- projections ----
            def proj_v():
                for lc in range(0, LK, 512):
                    w = min(512, LK - lc)
                    xcv = get_xv(lc, w)
                    for t4 in range((w + 127) // 128):
                        t = lc // 128 + t4
                        ps = pj_pool.tile([128, 512], f32, tag="pj")
                        for dt in range(ND):
                            nc.tensor.matmul(
                                ps[:, :EH],
                                lhsT=xcv[:, dt * 512 + t4 * 128 : dt * 512 + (t4 + 1) * 128],
                                rhs=wv_sb[:, dt * EH : (dt + 1) * EH],
                                start=(dt == 0),
                                stop=(dt == ND - 1),
                            )
                        # mask rows past V_len (per-partition key mask) so
                        # masked keys contribute exactly zero to the numerator
                        nc.scalar.mul(
                            v4[:, t, :, 0:DH],
                            ps[:, :EH].rearrange("p (h e) -> p h e", h=NH, e=DH),
                            kmf[:, t : t + 1],
                        )

            def proj_kq(eb):
                for lc in range(0, LK, 512):
                    w = min(512, LK - lc)
                    xck = get_xk(lc, w)
                    ps = pj_pool.tile([128, 512], f32, tag="pj")
                    for dt in range(ND):
                        nc.tensor.matmul(
                            ps[:, :w],
                            lhsT=wk_sb[:, dt * EH + eb * 128 : dt * EH + (eb + 1) * 128],
                            rhs=xck[:, dt * 512 : dt * 512 + w],
                            start=(dt == 0),
                            stop=(dt == ND - 1),
                        )
                    nc.vector.tensor_copy(
                        kt_sb[:, eb * LKS + lc : eb * LKS + lc + w], ps[:, :w]
                    )
                for lc in range(0, LQ, 512):
                    w = min(512, LQ - lc)
                    xcq = get_xq(lc, w)
                    ps = pj_pool.tile([128, 512], f32, tag="pj")
                    for dt in range(ND):
                        nc.tensor.matmul(
                            ps[:, :w],
                            lhsT=wq_sb[:, dt * EH + eb * 128 : dt * EH + (eb + 1) * 128],
                            rhs=xcq[:, dt * 512 : dt * 512 + w],
                            start=(dt == 0),
                            stop=(dt == ND - 1),
                        )
                    nc.vector.tensor_copy(
                        qt_sb[:, eb * LQS + lc : eb * LQS + lc + w], ps[:, :w]
                    )

            # ---- attention; projection of the NEXT head pair interleaved ----
            proj_kq(0)
            proj_v()
            for hp in range(NEB):
                hA, hB = 2 * hp, 2 * hp + 1
                for lqs in range(0, LQ, 256):
                    w = min(256, LQ - lqs)
                    nlqb = w // 128
                    tA = t_pool.tile([128, NTK * 256], bf16, tag="t")
                    tB = t_pool.tile([128, NTK * 256], bf16, tag="t")
                    for (t0, tn) in quads:
                        psA = pp_pool.tile([128, 1024], f32, tag="sq")
                        psB = pp_pool.tile([128, 1024], f32, tag="sq")
                        for j in range(tn):
                            tt = t0 + j
                            nc.tensor.matmul(
                                psA[:, j * w : (j + 1) * w],
                                lhsT=kt_sb[0:64, hp * LKS + tt * 128 : hp * LKS + (tt + 1) * 128],
                                rhs=qt_sb[0:64, hp * LQS + lqs : hp * LQS + lqs + w],
                                start=True,
                                stop=True,
                            )
                            nc.tensor.matmul(
                                psB[:, j * w : (j + 1) * w],
                                lhsT=kt_sb[64:128, hp * LKS + tt * 128 : hp * LKS + (tt + 1) * 128],
                                rhs=qt_sb[64:128, hp * LQS + lqs : hp * LQS + lqs + w],
                                start=True,
                                stop=True,
                            )
                        w_all = tn * w
                        nc.scalar.activation(
                            tA[:, t0 * w : t0 * w + w_all], psA[:, :w_all],
                            mybir.ActivationFunctionType.Exp,
                        )
                        nc.scalar.activation(
                            tB[:, t0 * w : t0 * w + w_all], psB[:, :w_all],
                            mybir.ActivationFunctionType.Exp,
                        )
                    for lb in range(nlqb):
                        pavA = pav_pool.tile([128, VW], f32, tag="av")
                        pavB = pav_pool.tile([128, VW], f32, tag="av")
                        for tt in range(NTK):
                            nc.tensor.matmul(
                                pavA[:, 0:VW],
                                lhsT=tA[:, tt * w + lb * 128 : tt * w + lb * 128 + 128],
                                rhs=v4[:, tt, hA, :],
                                start=(tt == 0),
                                stop=(tt == NTK - 1),
                            )
                            nc.tensor.matmul(
                                pavB[:, 0:VW],
                                lhsT=tB[:, tt * w + lb * 128 : tt * w + lb * 128 + 128],
                                rhs=v4[:, tt, hB, :],
                                start=(tt == 0),
                                stop=(tt == NTK - 1),
                            )
                        rA = r_pool.tile([128, 1], f32, tag="r")
                        rB = r_pool.tile([128, 1], f32, tag="r")
                        nc.vector.reciprocal(rA[:, :], pavA[:, DH : DH + 1])
                        nc.vector.reciprocal(rB[:, :], pavB[:, DH : DH + 1])
                        oA = o_pool.tile([128, DH], fp16, tag="o")
                        oB = o_pool.tile([128, DH], fp16, tag="o")
                        nc.scalar.mul(oA[:, :], pavA[:, 0:DH], rA[:, 0:1])
                        nc.scalar.mul(oB[:, :], pavB[:, 0:DH], rB[:, 0:1])
                        ls = lqs + lb * 128
                        nc.sync.dma_start(
                            xo[ls : ls + 128, hA * DH : (hA + 1) * DH], oA[:, :]
                        )
                        nc.sync.dma_start(
                            xo[ls : ls + 128, hB * DH : (hB + 1) * DH], oB[:, :]
                        )
                if hp + 1 < NEB:
                    proj_kq(hp + 1)

            # ---- pack the output stream ----
            # Each core writes its result into every candidate slot, scaled
            # by the one-hot core indicator (data-routing again: SPMD cores
            # can't address by core id). ReduceScatter(add) then leaves core
            # c exactly slot rows [c*SR, (c+1)*SR).
            for cc in range(8):
                rows = min(LQ, core_lq[cc])
                for ls in range(0, rows, 128):
                    h = min(128, rows - ls)
                    ot = mat_pool.tile([128, EH], fp16, tag="ot")
                    nc.sync.dma_start(ot[:h, :], xo[ls : ls + h, :])
                    om = mat_pool.tile([128, EH], fp16, tag="om")
                    nc.vector.tensor_scalar_mul(
                        om[:h, :], ot[:h, :], sel_sb[:h, 8 + cc : 9 + cc]
                    )
                    nc.sync.dma_start(
                        contrib[S[cc] + ls : S[cc] + ls + h, :], om[:h, :]
                    )
            if STREAM < 8 * SR:
                zt = win_pool.tile([128, EH], fp16, tag="zt")
                nc.vector.memset(zt[:], 0.0)
                for r0 in range(STREAM, 8 * SR, 128):
                    h = min(128, 8 * SR - r0)
                    nc.sync.dma_start(contrib[r0 : r0 + h, :], zt[:h, :])
            nc.gpsimd.collective_compute(
                "ReduceScatter",
                mybir.AluOpType.add,
                replica_groups=[list(range(8))],
                ins=[contrib[:].opt()],
                outs=[rsout[:].opt()],
            )
            nc.gpsimd.dma_start(outp, rsout[:])

    nc.compile()
    return nc


def _get_nc(cfg):
    key = (cfg["LQ"], cfg["LK"], cfg["Qe"], cfg["Ke"])
    if key not in _nc_cache:
        _nc_cache[key] = _build(cfg)
    return _nc_cache[key]


def kernel(Q_seq, K_seq, V_seq, Q_len, V_len, WQ, WK, WV):
    _setup_jax_cache()
    from concourse.bass_utils import run_bass_kernel_spmd

    Q_seq = np.asarray(Q_seq, np.float32)
    K_seq = np.asarray(K_seq, np.float32)
    V_seq = np.asarray(V_seq, np.float32)
    WQ = np.asarray(WQ, np.float32)
    WK = np.asarray(WK, np.float32)
    WV = np.asarray(WV, np.float32)
    q_len = np.asarray(Q_len).reshape(-1).astype(np.int64)
    v_len = np.asarray(V_len).reshape(-1).astype(np.int64)
    assert len(q_len) == B and Q_seq.shape == (B, L, D)

    # V_len == 0 masks every key, which softmax's shift-invariance turns
    # into "no mask"; Q rows past Q_len are zeroed host-side.
    vl = [int(v) if v > 0 else L for v in v_len]
    Qe = tuple(min(int(q), L) for q in q_len)
    Ke = tuple(min(v, L) for v in vl)
    LQ, LK = _ceil128(max(Qe)), _ceil128(max(Ke))
    out = np.zeros((B, L, H * DH), np.float32)
    if LQ == 0:
        return out
    NTK = LK // 128
    cfg = {"LQ": LQ, "LK": LK, "Qe": Qe, "Ke": Ke}
    off, TOT, CH = _layout(Qe, Ke)
    nc = _get_nc(cfg)

    # ---- pack the upload stream (each byte uploaded exactly once) ----
    f16 = np.float16
    bf16 = ml_dtypes.bfloat16
    X = np.zeros((D, 8 * CH), f16)
    for b in range(B):
        if Qe[b]:
            X[:, off["q"][b] : off["q"][b] + Qe[b]] = Q_seq[b, : Qe[b]].astype(f16).T
        X[:, off["k"][b] : off["k"][b] + Ke[b]] = K_seq[b, : Ke[b]].astype(f16).T
        X[:, off["v"][b] : off["v"][b] + Ke[b]] = V_seq[b, : Ke[b]].astype(f16).T
    X[:, off["wq"] : off["wq"] + D] = WQ
    X[:, off["wk"] : off["wk"] + D] = WK
    X[:, off["wv"] : off["wv"] + D] = WV
    # contiguous per-core chunks so the concatenate inside
    # run_bass_via_pjrt is a plain memcpy, not a strided gather
    Xc = [np.ascontiguousarray(X[:, c * CH : (c + 1) * CH]) for c in range(8)]

    in_maps = []
    core_meta = []
    for b in range(B):
        for hg in range(2):
            c = 2 * b + hg
            s = np.zeros((128, 24), np.float32)
            # reference semantics for V_len==0: scores-1e12 underflows all
            # scores equally in fp32, so softmax is UNIFORM over all keys.
            # Zeroing q reproduces that exactly (exp(0)=1 for every key).
            s[:, b] = 0.0 if int(v_len[b]) == 0 else 1.0
            s[:, 16 + b] = 1.0
            s[:, 4 + hg] = 1.0
            s[:, 8 + c] = 1.0
            kmask = (np.arange(LK) < vl[b]).astype(np.float32)
            kmv = np.repeat(
                kmask.reshape(NTK, 128).T[:, :, None], NH, axis=2
            ).reshape(128, NTK * NH)
            in_maps.append({
                "xin": Xc[c],
                "sel": s,
                "km": kmv.astype(bf16),
            })
            core_meta.append((b, hg))

    import time as _time

    trace = os.environ.get("NN_ATT_TRACE") == "1"
    t_spmd = _time.time()
    try:
        res = run_bass_kernel_spmd(
            nc, in_maps, core_ids=list(range(8)), trace=trace,
            **({"trace_cores": list(range(8))} if trace else {}),
        )
    except Exception:
        if not trace:
            raise
        res = run_bass_kernel_spmd(nc, in_maps, core_ids=list(range(8)))
    global LAST_EXEC_NS, LAST_RESULT, LAST_SPMD_WALL_NS
    LAST_SPMD_WALL_NS = int((_time.time() - t_spmd) * 1e9)
    LAST_RESULT = res
    if res.exec_time_ns:
        LAST_EXEC_NS = int(res.exec_time_ns)

    stream = np.concatenate([res.results[c]["outp"] for c in range(8)], axis=0)
    S = 0
    for c, (b, hg) in enumerate(core_meta):
        blk = Qe[b]
        nq = min(int(q_len[b]), LQ, L)
        if nq > 0:
            out[b, :nq, hg * EH : (hg + 1) * EH] = stream[S : S + nq].astype(
                np.float32
            )
        S += blk
    return out
